# revision 3
# baseline (speedup 1.0000x reference)
"""Trainium2 Bass kernel for nn_CustomCellModel (dual-cell RNN over T=512).

Sharding: 8 cores, 4 per cell (cell0=tanh, cell1=relu), batch-sharded
16 rows/core, no collectives. Per core: indirect-DMA embedding gather ->
PE transpose -> xeT projection (f32r matmuls) into an SBUF-resident
X[128, T, 4, 16] fp32 tile, then the sequential scan (bf16 Wh matmuls
into 2 PSUM groups + DVE/ACT/GPSIMD chain with per-core tanh/relu masks),
then a partial FC. Host sums the two cell partials.
"""

import numpy as np
import ml_dtypes

B, T, V, E, H = 64, 512, 32000, 256, 512
BL = 16          # batch rows per core
NCORES = 8
USE_F32R = True

_compiled = None


def _build(t_steps):
    import concourse.bass as bass
    import concourse.tile as tile
    from concourse import bacc, mybir
    from concourse.masks import make_identity

    f32 = mybir.dt.float32
    bf16 = mybir.dt.bfloat16
    NT128 = t_steps * BL // 128       # gather tiles of 128 tokens
    NT512 = NT128 // 4                # groups of 512 tokens (= 32 t-steps)

    nc = bacc.Bacc("TRN2", debug=False, num_devices=NCORES)
    emb_t = nc.dram_tensor("emb_table", [V, E], f32, kind="ExternalInput").ap()
    idx_t = nc.dram_tensor("idx", [128, NT128], mybir.dt.int32, kind="ExternalInput").ap()
    wx_dt0 = mybir.dt.float32r if USE_F32R else f32
    wx_t = nc.dram_tensor("wx", [E, H], wx_dt0, kind="ExternalInput").ap()
    wh_t = nc.dram_tensor("wh", [H, H], bf16, kind="ExternalInput").ap()
    b_t = nc.dram_tensor("bias", [H], f32, kind="ExternalInput").ap()
    ab_t = nc.dram_tensor("ab", [128, 2], f32, kind="ExternalInput").ap()
    fc_t = nc.dram_tensor("fc", [H], f32, kind="ExternalInput").ap()
    out_t = nc.dram_tensor("out", [1, BL], f32, kind="ExternalOutput").ap()

    with tile.TileContext(nc) as tc:
        with (
            tc.tile_pool(name="const", bufs=1) as cp,
            tc.tile_pool(name="gp", bufs=6) as gp,
            tc.tile_pool(name="etp", bufs=2) as etp,
            tc.tile_pool(name="zp", bufs=3) as zp,
            tc.tile_pool(name="hp", bufs=3) as hp,
            tc.tile_pool(name="trp", bufs=1, space="PSUM") as trp,
            tc.tile_pool(name="xep", bufs=2, space="PSUM") as xep,
            tc.tile_pool(name="zap", bufs=2, space="PSUM") as zap,
            tc.tile_pool(name="zbp", bufs=2, space="PSUM") as zbp,
        ):
            # ---- constants into SBUF ----
            idx_sb = cp.tile([128, NT128], mybir.dt.int32)
            nc.sync.dma_start(out=idx_sb[:], in_=idx_t[:])
            wx_dt = wx_dt0
            wx_sb = cp.tile([128, 2, H], wx_dt)        # [p, kE, h]
            nc.sync.dma_start(out=wx_sb[:],
                              in_=wx_t.rearrange("(k p) h -> p k h", p=128))
            wh_sb = cp.tile([128, 4, 4, 128], bf16)    # [p, kH, mH, q]
            nc.sync.dma_start(out=wh_sb[:], in_=wh_t.rearrange("(k p) (m q) -> p k m q", p=128, q=128))
            b_sb = cp.tile([128, 4], f32)
            nc.sync.dma_start(out=b_sb[:], in_=b_t.rearrange("(m p) -> p m", p=128))
            ab_sb = cp.tile([128, 2], f32)
            nc.sync.dma_start(out=ab_sb[:], in_=ab_t[:])
            fc_sb = cp.tile([128, 4], f32)
            nc.sync.dma_start(out=fc_sb[:], in_=fc_t.rearrange("(m p) -> p m", p=128))
            ident = cp.tile([128, 128], f32)
            make_identity(nc, ident[:])
            X = cp.tile([128, t_steps, 4, BL], f32)    # xe, H-chunk on partitions

            h_cur = hp.tile([128, 4, BL], bf16, name="h")
            nc.vector.memset(h_cur[:], 0.0)
            h_fin = None

            def phase_a_tile(j4):
                embTs = []
                for e in range(2):
                    embT = etp.tile([128, 512], wx_dt, name=f"embT{e}")
                    embTs.append(embT)
                for s in range(4):
                    g = gp.tile([128, E], f32, name="g")
                    nc.gpsimd.indirect_dma_start(
                        out=g[:], out_offset=None, in_=emb_t[:],
                        in_offset=bass.IndirectOffsetOnAxis(
                            ap=idx_sb[:, j4 * 4 + s:j4 * 4 + s + 1], axis=0))
                    for e in range(2):
                        ptr = trp.tile([128, 128], f32, name="ptr")
                        nc.tensor.transpose(out=ptr[:], in_=g[:, e * 128:(e + 1) * 128],
                                            identity=ident[:])
                        nc.vector.tensor_copy(out=embTs[e][:, s * 128:(s + 1) * 128], in_=ptr[:])
                for m in range(4):
                    pxe = xep.tile([128, 512], f32, name="pxe")
                    for k in range(2):
                        nc.tensor.matmul(out=pxe[:], lhsT=wx_sb[:, k, m * 128:(m + 1) * 128],
                                         rhs=embTs[k][:], start=(k == 0), stop=(k == 1))
                    nc.scalar.activation(
                        out=X[:, j4 * 32:(j4 + 1) * 32, m, :],
                        in_=pxe[:].rearrange("p (t b) -> p t b", b=BL),
                        func=mybir.ActivationFunctionType.Identity,
                        bias=b_sb[:, m:m + 1], scale=1.0)

            def scan_step(t):
                nonlocal h_cur, h_fin
                ZA = zap.tile([128, 2, BL], f32, name="ZA")
                ZB = zbp.tile([128, 2, BL], f32, name="ZB")
                for m in range(4):
                    Z = ZA if m < 2 else ZB
                    for k in range(4):
                        nc.tensor.matmul(out=Z[:, m % 2, :], lhsT=wh_sb[:, k, m, :],
                                         rhs=h_cur[:, k, :], start=(k == 0), stop=(k == 3))
                last = (t == t_steps - 1)
                if last:
                    h_fin = hp.tile([128, 4, BL], f32, name="hf")
                else:
                    h_next = hp.tile([128, 4, BL], bf16, name="h")
                for gidx, Z in ((0, ZA), (1, ZB)):
                    zg = zp.tile([128, 2, BL], f32, name="zg")
                    nc.vector.tensor_tensor(out=zg[:], in0=Z[:],
                                            in1=X[:, t, 2 * gidx:2 * gidx + 2, :],
                                            op=mybir.AluOpType.add)
                    tg = zp.tile([128, 2, BL], f32, name="tg")
                    nc.scalar.activation(out=tg[:], in_=zg[:],
                                         func=mybir.ActivationFunctionType.Tanh,
                                         scale=ab_sb[:, 0:1])
                    rg = zp.tile([128, 2, BL], f32, name="rg")
                    nc.vector.tensor_scalar(out=rg[:], in0=zg[:], scalar1=ab_sb[:, 1:2],
                                            scalar2=0.0, op0=mybir.AluOpType.mult,
                                            op1=mybir.AluOpType.max)
                    dst = h_fin if last else h_next
                    nc.gpsimd.tensor_tensor(out=dst[:, 2 * gidx:2 * gidx + 2, :],
                                            in0=tg[:], in1=rg[:], op=mybir.AluOpType.add)
                if not last:
                    h_cur = h_next

            # interleave: emit phase-A tile j, then scan steps of tile j-1
            for j4 in range(NT512):
                phase_a_tile(j4)
                if j4 > 0:
                    for t in range((j4 - 1) * 32, j4 * 32):
                        scan_step(t)
            for t in range((NT512 - 1) * 32, t_steps):
                scan_step(t)

            # ---- partial FC ----
            with tc.tile_pool(name="fcp", bufs=1, space="PSUM") as fcp:
                pfc = fcp.tile([1, BL], f32)
                for c in range(4):
                    nc.tensor.matmul(out=pfc[:], lhsT=fc_sb[:, c:c + 1],
                                     rhs=h_fin[:, c, :], start=(c == 0), stop=(c == 3))
                ob = zp.tile([1, BL], f32, name="ob")
                nc.vector.tensor_copy(out=ob[:], in_=pfc[:])
                nc.sync.dma_start(out=out_t[:], in_=ob[:])

    nc.compile()
    return nc


def _prep_inputs(x, emb_table, Wx0, Wh0, b0, Wx1, Wh1, b1, fc_w, fc_b, t_steps):
    x = np.asarray(x).astype(np.int32)
    emb_table = np.ascontiguousarray(np.asarray(emb_table, np.float32))
    fc_w = np.asarray(fc_w, np.float32).reshape(-1)
    cells = [
        (np.asarray(Wx0, np.float32), np.asarray(Wh0, np.float32),
         np.asarray(b0, np.float32), fc_w[:H], 1.0, 0.0),
        (np.asarray(Wx1, np.float32), np.asarray(Wh1, np.float32),
         np.asarray(b1, np.float32), fc_w[H:], 0.0, 1.0),
    ]
    NT128 = t_steps * BL // 128
    in_maps = []
    for c in range(NCORES):
        cell = c // 4
        brow0 = BL * (c % 4)
        Wx, Wh, bb, fch, a, bm = cells[cell]
        rows = x[brow0:brow0 + BL, :t_steps]          # [BL, T]
        idx_flat = rows.T.reshape(-1)                  # j = t*BL + bl
        idx_sb = idx_flat.reshape(NT128, 128).T.copy()  # [128, NT128]
        ab = np.zeros((128, 2), np.float32)
        ab[:, 0] = a
        ab[:, 1] = bm
        in_maps.append({
            "emb_table": emb_table,
            "idx": np.ascontiguousarray(idx_sb),
            "wx": Wx,
            "wh": Wh.astype(ml_dtypes.bfloat16),
            "bias": bb,
            "ab": ab,
            "fc": np.ascontiguousarray(fch),
        })
    return in_maps


def run(t_steps, trace=False, **inputs):
    """Build (cached), run on 8 cores, return (out[B], exec_time_ns)."""
    global _compiled
    from concourse.bass_utils import run_bass_kernel_spmd
    if _compiled is None or _compiled[0] != t_steps:
        _compiled = (t_steps, _build(t_steps))
    nc = _compiled[1]
    in_maps = _prep_inputs(t_steps=t_steps, **inputs)
    res = run_bass_kernel_spmd(nc, in_maps, core_ids=list(range(NCORES)), trace=trace)
    fc_b = np.asarray(inputs["fc_b"], np.float32).reshape(-1)
    out = np.zeros(B, np.float32)
    for q in range(4):
        p0 = res.results[q]["out"].reshape(BL)
        p1 = res.results[q + 4]["out"].reshape(BL)
        out[BL * q:BL * (q + 1)] = p0 + p1 + fc_b[0]
    return out, res


def kernel(**inputs) -> np.ndarray:
    out, _ = run(T, trace=False, **inputs)
    return out
